# revision 1
# baseline (speedup 1.0000x reference)
"""Trainium2 Bass kernel for nn_GPTrack2D (dense transformer with linear
attention and a per-frame recurrence over L).

Sharding: batch (2) -> two groups of 4 cores; tokens (1024 -> 256/core)
within each group. Linear attention's k^T v state is all-reduced per frame
within the group; the all-reduce hides behind the previous frame's MLP
(software-pipelined emission). All activations are kept feature-major on
chip (D on partitions); the host pre-transposes inputs and post-transposes
outputs, and folds LN gains/biases into the weights.

Precision: the residual stream / carry / LN inputs are fp32 (activation
magnitudes reach ~1e5-1e6, beyond fp16 range). LN stats run as float32r
ones-matmuls (full PE rate at free-dim>=256). Only bounded normalized
quantities are fp16 matmul operands: z (LN outputs), q~, k~, v, gelu(y1),
and the kv state scaled by 1/256 (rescaled on consumption).
"""

import functools

import numpy as np

import concourse.bacc as bacc
import concourse.mybir as mybir
from concourse import tile
from concourse.bass_utils import run_bass_kernel_spmd

F32 = mybir.dt.float32
BF16 = mybir.dt.bfloat16
F16 = mybir.dt.float16
AF = mybir.ActivationFunctionType
ALU = mybir.AluOpType

B, L, N, D, M, H = 2, 12, 1024, 768, 3072, 12
NCORES = 8
GROUP = 4                 # cores per batch group
TOK = N // GROUP          # 256 tokens per core
KT = D // 128             # 6 d-tiles
MT = M // 128             # 24 m-tiles
F3 = 3 * D                # 2304
EPS = 1e-5
KVS = 1.0 / 256.0         # kv-state scale so fp16 holds it
KVSI = 256.0

DEBUG_H = False

# dev-scale knobs (full problem: L_RUN=12, LAYERS_RUN=2, DIRS_RUN=(0, 1))
L_RUN = L
LAYERS_RUN = 2
DIRS_RUN = (0, 1)

REPLICA_GROUPS = [[0, 1, 2, 3], [4, 5, 6, 7]]


# ---------------------------------------------------------------- host prep

def _pack_weights(inputs, dtype=np.float16):
    """Fold LN gains/biases into weights; tile for contiguous DMA."""
    segs = []
    for layer in range(LAYERS_RUN):
        for d in DIRS_RUN:
            gi = np.asarray(inputs["lni_g"][d, layer]); bi = np.asarray(inputs["lni_b"][d, layer])
            gh = np.asarray(inputs["lnh_g"][d, layer]); bh = np.asarray(inputs["lnh_b"][d, layer])
            go = np.asarray(inputs["lno_g"][d, layer]); bo = np.asarray(inputs["lno_b"][d, layer])
            Wqkv = np.asarray(inputs["Wqkv"][d, layer]); bqkv = np.asarray(inputs["bqkv"][d, layer])
            Wqkvh = np.asarray(inputs["Wqkvh"][d, layer]); bqkvh = np.asarray(inputs["bqkvh"][d, layer])
            Wout = np.asarray(inputs["Wout"][d, layer]); bout = np.asarray(inputs["bout"][d, layer])
            W1 = np.asarray(inputs["W1"][d, layer]); b1 = np.asarray(inputs["b1"][d, layer])
            W2 = np.asarray(inputs["W2"][d, layer]); b2 = np.asarray(inputs["b2"][d, layer])

            gqkv = gi[:, None] * Wqkv                      # (D, 3D)
            gqkvh = gh[:, None] * Wqkvh
            cqkv = bi @ Wqkv + bqkv + bh @ Wqkvh + bqkvh   # (3D,)
            g1 = go[:, None] * W1                          # (D, M)
            c1 = bo @ W1 + b1                              # (M,)

            seg = dict(
                # (128, KT, F3): [p, kd, f] = gqkv[kd*128+p, f]
                gqkv=np.ascontiguousarray(
                    gqkv.reshape(KT, 128, F3).transpose(1, 0, 2)).astype(dtype),
                gqkvh=np.ascontiguousarray(
                    gqkvh.reshape(KT, 128, F3).transpose(1, 0, 2)).astype(dtype),
                cqkv=cqkv.reshape(1, F3).astype(dtype),
                wout=np.ascontiguousarray(
                    Wout.reshape(KT, 128, D).transpose(1, 0, 2)).astype(dtype),
                # bout scaled by KVS: the attn matmul result carries a KVS
                # factor and (attn_s + bout*KVS) is rescaled by KVSI on use
                bout=np.ascontiguousarray(
                    (bout * KVS).reshape(KT, 128).T).astype(np.float32),
                # (MT, 128, KT, 128): [mj, p, kd, f] = g1[kd*128+p, mj*128+f]
                g1=np.ascontiguousarray(
                    g1.reshape(KT, 128, MT, 128).transpose(2, 1, 0, 3)).astype(dtype),
                c1=np.ascontiguousarray(
                    c1.reshape(MT, 128).T).astype(np.float32),    # (128, MT)
                w2=W2.reshape(MT, 128, D).astype(dtype),          # (MT, 128, D)
                b2=np.ascontiguousarray(
                    b2.reshape(KT, 128).T).astype(np.float32),    # (128, KT)
            )
            segs.append(seg)
    return segs


def _feat_major(a, dtype):
    """(..., tok, D) -> (..., 128, KT, tok) tiled feature-major."""
    t = np.moveaxis(np.asarray(a), -1, -2)                # (..., D, tok)
    shp = t.shape[:-2]
    t = t.reshape(shp + (KT, 128, t.shape[-1]))           # (..., KT, 128, tok)
    t = np.moveaxis(t, -3, -2)                            # (..., 128, KT, tok)
    return np.ascontiguousarray(t).astype(dtype)


def make_in_maps(inputs):
    segs = _pack_weights(inputs)
    in_maps = []
    for core in range(NCORES):
        b = core // GROUP
        s = (core % GROUP) * TOK
        m = {}
        m["x_in"] = _feat_major(
            np.asarray(inputs["x"])[b, :L_RUN, s:s + TOK, :], np.float32)
        m["h0_in"] = _feat_major(
            np.asarray(inputs["hidden"])[b, s:s + TOK, :], np.float32)
        m["spat"] = _feat_major(
            np.asarray(inputs["spatial_pos"])[b, s:s + TOK, :], np.float32)
        tp = np.asarray(inputs["temporal_pos"])[b, :L_RUN, :]   # (L, D)
        tp = tp.T.reshape(KT, 128, L_RUN).transpose(1, 0, 2)
        m["tpos"] = np.ascontiguousarray(tp).astype(np.float32)  # (128, KT, L)
        for si, seg in enumerate(segs):
            for k, v in seg.items():
                m[f"{k}_{si}"] = v
        in_maps.append(m)
    return in_maps


def unshard_output(results):
    """results: per-core dicts with 'out_x' (L_RUN, 128, KT, TOK) f32."""
    out = np.empty((B, L_RUN, N, D), np.float32)
    for core in range(NCORES):
        b = core // GROUP
        s = (core % GROUP) * TOK
        o = np.asarray(results[core]["out_x"])            # (L, 128, KT, TOK)
        o = o.transpose(0, 2, 1, 3).reshape(L_RUN, D, TOK)
        out[b, :, s:s + TOK, :] = np.moveaxis(o, -1, -2)
    return out


# ---------------------------------------------------------------- kernel build

class Ctx:
    """Pools, constants and persistent tiles used during emission."""


def _layer_norm(nc, cx, src32, tag):
    """Feature-major LN stats for an SBUF (128, KT, TOK) f32 tile.

    Returns (rb, mrb) f32 (128, TOK) broadcast tiles: z = src*rb - mrb.
    Sums over the partition (feature) axis via all-ones f32r matmuls.
    """
    s1 = cx.psA.tile([128, TOK], F32, name="ps", tag="ps")
    s2 = cx.psA.tile([128, TOK], F32, name="ps", tag="ps")
    for kd in range(KT):
        xb = cx.tmp.tile([128, TOK], BF16, name="xb", tag="xb")
        nc.vector.tensor_copy(xb[:], src32[:, kd, :])
        sq = cx.tmp.tile([128, TOK], BF16, name="sq", tag="sq")
        nc.scalar.activation(sq[:], src32[:, kd, :], AF.Square)
        nc.tensor.matmul(s1[:], cx.onesB[:], xb[:],
                         start=(kd == 0), stop=(kd == KT - 1))
        nc.tensor.matmul(s2[:], cx.onesB[:], sq[:],
                         start=(kd == 0), stop=(kd == KT - 1))
    mean = cx.tmp.tile([128, TOK], F32, name="mean", tag="mean")
    nc.vector.tensor_scalar_mul(mean[:], s1[:], 1.0 / D)
    msq = cx.tmp.tile([128, TOK], F32, name="msq", tag="msq")
    nc.vector.tensor_mul(msq[:], mean[:], mean[:])
    ve = cx.tmp.tile([128, TOK], F32, name="ve", tag="ve")
    nc.vector.scalar_tensor_tensor(ve[:], s2[:], 1.0 / D, msq[:],
                                   op0=ALU.mult, op1=ALU.subtract)
    sd = cx.tmp.tile([128, TOK], F32, name="sd", tag="sd")
    nc.scalar.activation(sd[:], ve[:], AF.Sqrt, bias=cx.epsc[:])
    rb = cx.tmp.tile([128, TOK], F32, name=f"rb_{tag}", tag=f"rb_{tag}")
    nc.vector.reciprocal(rb[:], sd[:])
    mrb = cx.tmp.tile([128, TOK], F32, name=f"mrb_{tag}", tag=f"mrb_{tag}")
    nc.vector.tensor_mul(mrb[:], mean[:], rb[:])
    return rb, mrb


def _normalize(nc, cx, pool, src32, rb, mrb, tag):
    """z[kd] = src*rb - mrb -> fp16 (128, KT, TOK) tile."""
    z = pool.tile([128, KT, TOK], F16, name=f"z_{tag}", tag=f"z_{tag}")
    for kd in range(KT):
        t = cx.tmp.tile([128, TOK], F32, name="zt", tag="zt")
        nc.vector.tensor_mul(t[:], src32[:, kd, :], rb[:])
        nc.vector.tensor_sub(z[:, kd, :], t[:], mrb[:])
    return z


def _elu1(nc, cx, psum_ap, out_ap, ncols):
    """out = elu(psum)+1 = exp(min(x,0)) + max(x,0)."""
    tmin = cx.tmp.tile([128, 384], F32, name="emin", tag="emin")
    texp = cx.tmp.tile([128, 384], F32, name="eexp", tag="eexp")
    nc.vector.tensor_scalar_min(tmin[:, :ncols], psum_ap, 0.0)
    nc.scalar.activation(texp[:, :ncols], tmin[:, :ncols], AF.Exp)
    nc.vector.scalar_tensor_tensor(out_ap, psum_ap, 0.0, texp[:, :ncols],
                                   op0=ALU.max, op1=ALU.add)


def build_nc():
    nc = bacc.Bacc("TRN2", target_bir_lowering=False, debug=False,
                   num_devices=NCORES)

    x_in = nc.dram_tensor("x_in", [L_RUN, 128, KT, TOK], F32, kind="ExternalInput")
    h0_in = nc.dram_tensor("h0_in", [128, KT, TOK], F32, kind="ExternalInput")
    spat = nc.dram_tensor("spat", [128, KT, TOK], F32, kind="ExternalInput")
    tpos = nc.dram_tensor("tpos", [128, KT, L_RUN], F32, kind="ExternalInput")
    nseg = LAYERS_RUN * len(DIRS_RUN)
    segs = []
    for si in range(nseg):
        segs.append(dict(
            gqkv=nc.dram_tensor(f"gqkv_{si}", [128, KT, F3], F16, kind="ExternalInput"),
            gqkvh=nc.dram_tensor(f"gqkvh_{si}", [128, KT, F3], F16, kind="ExternalInput"),
            cqkv=nc.dram_tensor(f"cqkv_{si}", [1, F3], F16, kind="ExternalInput"),
            wout=nc.dram_tensor(f"wout_{si}", [128, KT, D], F16, kind="ExternalInput"),
            bout=nc.dram_tensor(f"bout_{si}", [128, KT], F32, kind="ExternalInput"),
            g1=nc.dram_tensor(f"g1_{si}", [MT, 128, KT, 128], F16, kind="ExternalInput"),
            c1=nc.dram_tensor(f"c1_{si}", [128, MT], F32, kind="ExternalInput"),
            w2=nc.dram_tensor(f"w2_{si}", [MT, 128, D], F16, kind="ExternalInput"),
            b2=nc.dram_tensor(f"b2_{si}", [128, KT], F32, kind="ExternalInput"),
        ))
    out_x = nc.dram_tensor("out_x", [L_RUN, 128, KT, TOK], F32, kind="ExternalOutput")
    dbg_h = (nc.dram_tensor("dbg_h", [128, KT, TOK], F32, kind="ExternalOutput")
             if DEBUG_H else None)

    with tile.TileContext(nc) as tc:
        with (
            tc.tile_pool(name="cst", bufs=1) as cst,
            tc.tile_pool(name="wt", bufs=1) as wt,
            tc.tile_pool(name="stream", bufs=4) as stream,
            tc.tile_pool(name="act1", bufs=1) as act1,
            tc.tile_pool(name="act2", bufs=2) as act2,
            tc.tile_pool(name="state", bufs=1) as state,
            tc.tile_pool(name="tmp", bufs=1) as tmp,
            tc.tile_pool(name="psA", bufs=2, space="PSUM") as psA,
            tc.tile_pool(name="psY", bufs=6, space="PSUM") as psY,
            tc.tile_pool(name="dram", bufs=4, space="DRAM") as dram,
        ):
            cx = Ctx()
            cx.wt, cx.stream, cx.act1, cx.act2 = wt, stream, act1, act2
            cx.state, cx.tmp, cx.psA, cx.psY, cx.dram = state, tmp, psA, psY, dram

            cx.onesB = cst.tile([128, 128], BF16, name="onesB")
            nc.vector.memset(cx.onesB[:], 1.0)
            cx.ones1 = cst.tile([1, TOK], F16, name="ones1")
            nc.vector.memset(cx.ones1[:], 1.0)
            cx.epsc = cst.tile([128, 1], F32, name="epsc")
            nc.vector.memset(cx.epsc[:], EPS)
            cx.spat = cst.tile([128, KT, TOK], F32, name="spatc")
            nc.sync.dma_start(cx.spat[:], spat.ap())
            cx.tpos = cst.tile([128, KT, L_RUN], F32, name="tposc")
            nc.sync.dma_start(cx.tpos[:], tpos.ap())
            # block-diag kv holder: off-diagonal blocks stay zero forever
            cx.bd16 = state.tile([128, KT, 128], F16, name="bd16", tag="bd16")
            nc.vector.memset(cx.bd16[:], 0.0)

            x1_sc = dram.tile([L_RUN, 128, KT, TOK], F32, name="x1_sc", tag="x1_sc")
            yf_sc = dram.tile([L_RUN, 128, KT, TOK], F32, name="yf_sc", tag="yf_sc")

            for layer in range(LAYERS_RUN):
                x_src = x_in.ap() if layer == 0 else x1_sc
                last_layer = layer == LAYERS_RUN - 1
                for dir_i, d in enumerate(DIRS_RUN):
                    si = layer * len(DIRS_RUN) + dir_i
                    fwd = d == 0
                    last_scan = dir_i == len(DIRS_RUN) - 1
                    frames = (list(range(L_RUN)) if fwd
                              else list(range(L_RUN - 1, -1, -1)))
                    if not last_scan:
                        out_dst = yf_sc
                    elif last_layer:
                        out_dst = out_x.ap()
                    else:
                        out_dst = x1_sc
                    h32 = _emit_scan(nc, cx, segs[si], x_src, h0_in, frames,
                                     pos_fixed=(layer if fwd else None),
                                     yf_sc=yf_sc, fwd=fwd, out_dst=out_dst)
            if DEBUG_H:
                nc.sync.dma_start(dbg_h.ap(), h32[:])
    nc.compile()
    return nc


def _emit_scan(nc, cx, seg, x_src, h0_in, frames, pos_fixed, yf_sc, fwd,
               out_dst):
    w = {}
    for nm, shape, dt in (("gqkv", [128, KT, F3], F16),
                          ("gqkvh", [128, KT, F3], F16),
                          ("wout", [128, KT, D], F16),
                          ("cqkv", [1, F3], F16),
                          ("bout", [128, KT], F32),
                          ("c1", [128, MT], F32),
                          ("b2", [128, KT], F32)):
        w[nm] = cx.wt.tile(shape, dt, name=nm, tag=nm)
        nc.sync.dma_start(w[nm][:], seg[nm].ap())

    # h carry (f32), re-initialized from h0 each scan
    h32 = cx.state.tile([128, KT, TOK], F32, name="h32", tag="h32")
    nc.sync.dma_start(h32[:], h0_in.ap())

    pend = None
    for t in frames:
        pend = _emit_frame(nc, cx, seg, w, t, x_src, h32, pos_fixed, yf_sc,
                           fwd, out_dst, pend)
    _emit_mlp(nc, cx, seg, w, pend)
    return h32


def _emit_frame(nc, cx, seg, w, t, x_src, h32, pos_fixed, yf_sc, fwd,
                out_dst, pend):
    tp = pos_fixed if pos_fixed is not None else t

    # ---- load x_t; x_eff = x + pos_t (in place); h_eff = h + pos_tp
    x16 = cx.act2.tile([128, KT, TOK], F32, name="xe", tag="xe")
    nc.sync.dma_start(x16[:], x_src[t])
    heff = cx.act1.tile([128, KT, TOK], F32, name="heff", tag="heff")
    for kd in range(KT):
        nc.vector.scalar_tensor_tensor(
            x16[:, kd, :], cx.spat[:, kd, :], cx.tpos[:, kd, t:t + 1],
            x16[:, kd, :], op0=ALU.mult, op1=ALU.add)
        nc.vector.scalar_tensor_tensor(
            heff[:, kd, :], cx.spat[:, kd, :], cx.tpos[:, kd, tp:tp + 1],
            h32[:, kd, :], op0=ALU.mult, op1=ALU.add)
    xeff = x16

    # ---- layer norms + normalized activations (fp16)
    rb_x, mrb_x = _layer_norm(nc, cx, xeff, "x")
    zx = _normalize(nc, cx, cx.act1, xeff, rb_x, mrb_x, "x")
    rb_h, mrb_h = _layer_norm(nc, cx, heff, "h")
    zh = _normalize(nc, cx, cx.act1, heff, rb_h, mrb_h, "h")

    # ---- q (feature-major)
    q16 = cx.act1.tile([128, KT, TOK], F16, name="q16", tag="q16")
    for ft in range(KT):
        ps = cx.psA.tile([128, TOK], F32, name="ps", tag="ps")
        for kd in range(KT):
            nc.tensor.matmul(ps[:], w["gqkv"][:, kd, ft * 128:(ft + 1) * 128],
                             zx[:, kd, :], start=(kd == 0), stop=False)
        for kd in range(KT):
            nc.tensor.matmul(ps[:], w["gqkvh"][:, kd, ft * 128:(ft + 1) * 128],
                             zh[:, kd, :], start=False, stop=False)
        nc.tensor.matmul(ps[:], w["cqkv"][:, ft * 128:(ft + 1) * 128],
                         cx.ones1[:], start=False, stop=True)
        _elu1(nc, cx, ps[:], q16[:, ft, :], TOK)

    # ---- k, v (token-major): (128, 2, D) each [tok-half, feature]
    k16 = cx.act1.tile([128, 2, D], F16, name="k16", tag="k16")
    v16 = cx.act1.tile([128, 2, D], F16, name="v16", tag="v16")
    for tok2 in range(2):
        for fc in range(4):  # chunks of 384: k then v
            lo = D + fc * 384
            ps = cx.psA.tile([128, 384], F32, name="ps", tag="ps")
            for kd in range(KT):
                nc.tensor.matmul(ps[:], zx[:, kd, tok2 * 128:(tok2 + 1) * 128],
                                 w["gqkv"][:, kd, lo:lo + 384],
                                 start=(kd == 0), stop=False)
            for kd in range(KT):
                nc.tensor.matmul(ps[:], zh[:, kd, tok2 * 128:(tok2 + 1) * 128],
                                 w["gqkvh"][:, kd, lo:lo + 384],
                                 start=False, stop=False)
            nc.tensor.matmul(ps[:], cx.ones1[:, 0:128],
                             w["cqkv"][:, lo:lo + 384], start=False, stop=True)
            off = fc * 384
            if fc < 2:
                _elu1(nc, cx, ps[:], k16[:, tok2, off:off + 384], 384)
            else:
                nc.scalar.activation(v16[:, tok2, off - D:off - D + 384],
                                     ps[:], AF.Copy)

    # ---- kv state per head-pair; pack diag blocks into (128, 384) f32
    kvpack = cx.act1.tile([128, H * 32], F32, name="kvpack", tag="kvpack")
    for hp in range(KT):
        ps = cx.psA.tile([128, TOK], F32, name="ps", tag="ps")
        pskv = ps[:, 0:128]
        for tok2 in range(2):
            nc.tensor.matmul(pskv, k16[:, tok2, hp * 128:(hp + 1) * 128],
                             v16[:, tok2, hp * 128:(hp + 1) * 128],
                             start=(tok2 == 0), stop=(tok2 == 1))
        nc.vector.tensor_copy(kvpack[0:64, hp * 64:(hp + 1) * 64],
                              pskv[0:64, 0:64])
        nc.vector.tensor_copy(kvpack[64:128, hp * 64:(hp + 1) * 64],
                              pskv[64:128, 64:128])

    # ---- all-reduce kv within the token-shard group
    arin = cx.dram.tile([128, H * 32], F32, name="arin", tag="arin")
    arout = cx.dram.tile([128, H * 32], F32, name="arout", tag="arout")
    nc.sync.dma_start(arin[:], kvpack[:])
    nc.gpsimd.collective_compute(
        "AllReduce", ALU.add, replica_groups=REPLICA_GROUPS,
        ins=[arin.opt()], outs=[arout.opt()])

    # ---- deferred MLP of the previous frame (hides the all-reduce)
    if pend is not None:
        _emit_mlp(nc, cx, seg, w, pend)

    kvred = cx.act1.tile([128, H * 32], F32, name="kvred", tag="kvred")
    nc.sync.dma_start(kvred[:], arout[:])

    # ---- block-diag kv (fp16, scaled by KVS); o_s = blockdiag(kv_s) @ q
    for hp in range(KT):
        nc.vector.tensor_scalar_mul(cx.bd16[0:64, hp, 0:64],
                                    kvred[0:64, hp * 64:(hp + 1) * 64], KVS)
        nc.vector.tensor_scalar_mul(cx.bd16[64:128, hp, 64:128],
                                    kvred[64:128, hp * 64:(hp + 1) * 64], KVS)
    o16 = cx.act1.tile([128, KT, TOK], F16, name="o16", tag="o16")
    for hp in range(KT):
        ps = cx.psA.tile([128, TOK], F32, name="ps", tag="ps")
        nc.tensor.matmul(ps[:], cx.bd16[:, hp, :], q16[:, hp, :],
                         start=True, stop=True)
        nc.scalar.activation(o16[:, hp, :], ps[:], AF.Copy)

    # ---- attn (feature-major, scaled by KVS; rescale on consumption)
    x232 = cx.act2.tile([128, KT, TOK], F32, name="x232", tag="x232")
    for ft in range(KT):
        ps = cx.psA.tile([128, TOK], F32, name="ps", tag="ps")
        for hp in range(KT):
            nc.tensor.matmul(ps[:], w["wout"][:, hp, ft * 128:(ft + 1) * 128],
                             o16[:, hp, :], start=(hp == 0), stop=(hp == KT - 1))
        bo = w["bout"][:, ft:ft + 1]          # already scaled by KVS
        at = cx.tmp.tile([128, TOK], F32, name="at", tag="at")
        nc.vector.tensor_scalar(at[:], ps[:], bo, KVSI,
                                op0=ALU.add, op1=ALU.mult)  # attn = (ps+bo)*256
        # h_next = attn + (h + pos)
        ht = cx.tmp.tile([128, TOK], F32, name="ht", tag="ht")
        nc.vector.scalar_tensor_tensor(
            ht[:], cx.spat[:, ft, :], cx.tpos[:, ft, tp:tp + 1], h32[:, ft, :],
            op0=ALU.mult, op1=ALU.add)
        nc.vector.tensor_add(h32[:, ft, :], at[:], ht[:])
        # x2 = attn + x_eff
        nc.vector.tensor_add(x232[:, ft, :], at[:], xeff[:, ft, :])

    # ---- z2 for the deferred MLP
    rb2, mrb2 = _layer_norm(nc, cx, x232, "o")
    z2 = _normalize(nc, cx, cx.act2, x232, rb2, mrb2, "o")

    return dict(t=t, z2=z2, x232=x232, fwd=fwd, out_dst=out_dst, yf_sc=yf_sc)


def _emit_mlp(nc, cx, seg, w, pend):
    t, z2, x232 = pend["t"], pend["z2"], pend["x232"]
    fwd, out_dst, yf_sc = pend["fwd"], pend["out_dst"], pend["yf_sc"]

    # y1 = gelu(z2 @ G1 + c1), feature-major (m on partitions)
    y1g = cx.act1.tile([128, MT, TOK], F16, name="y1g", tag="y1g")
    for mj in range(MT):
        g1s = cx.stream.tile([128, KT, 128], F16, name="g1s", tag="g1s")
        nc.sync.dma_start(g1s[:], seg["g1"].ap()[mj])
        ps = cx.psA.tile([128, TOK], F32, name="ps", tag="ps")
        for kd in range(KT):
            nc.tensor.matmul(ps[:], g1s[:, kd, :], z2[:, kd, :],
                             start=(kd == 0), stop=(kd == KT - 1))
        nc.scalar.activation(y1g[:, mj, :], ps[:], AF.Gelu,
                             bias=w["c1"][:, mj:mj + 1])

    # y = y1g @ W2 (+ b2); out = x2 + y
    yps = [cx.psY.tile([128, TOK], F32, name="psy", tag="psy")
           for _ in range(KT)]
    for mj in range(MT):
        w2s = cx.stream.tile([128, D], F16, name="w2s", tag="w2s")
        nc.sync.dma_start(w2s[:], seg["w2"].ap()[mj])
        for ft in range(KT):
            nc.tensor.matmul(yps[ft][:], w2s[:, ft * 128:(ft + 1) * 128],
                             y1g[:, mj, :], start=(mj == 0), stop=(mj == MT - 1))

    outt = cx.act1.tile([128, KT, TOK], F32, name="outt", tag="outt")
    if fwd:
        for ft in range(KT):
            nc.vector.scalar_tensor_tensor(
                outt[:, ft, :], yps[ft][:], w["b2"][:, ft:ft + 1],
                x232[:, ft, :], op0=ALU.add, op1=ALU.add)
    else:
        yf = cx.act1.tile([128, KT, TOK], F32, name="yfld", tag="yfld")
        nc.sync.dma_start(yf[:], yf_sc[t])
        for ft in range(KT):
            yb = cx.tmp.tile([128, TOK], F32, name="yb", tag="yb")
            nc.vector.scalar_tensor_tensor(
                yb[:], yps[ft][:], w["b2"][:, ft:ft + 1], x232[:, ft, :],
                op0=ALU.add, op1=ALU.add)
            nc.vector.tensor_add(outt[:, ft, :], yb[:], yf[:, ft, :])
    nc.sync.dma_start(out_dst[t], outt[:])


# ---------------------------------------------------------------- entry point

@functools.cache
def _compiled_nc():
    return build_nc()


def kernel(**inputs):
    inputs = {k: np.asarray(v) for k, v in inputs.items()}
    nc = _compiled_nc()
    in_maps = make_in_maps(inputs)
    res = run_bass_kernel_spmd(nc, in_maps, list(range(NCORES)))
    return unshard_output(res.results)



# revision 9
# speedup vs baseline: 1.4948x; 1.4948x over previous
"""Trainium2 Bass kernel for nn_GPTrack2D (dense transformer with linear
attention and a per-frame recurrence over L).

Sharding: batch (2) -> two groups of 4 cores; tokens (1024 -> 256/core)
within each group. Linear attention's k^T v state is all-reduced per frame
within the group (fp16, scaled by 1/256; the 256 is folded into Wout);
the all-reduce hides behind the previous frame's MLP and the next frame's
x-side prefetch.

Per frame t (emission order):
  1. LN_h(t) stats (f32r/bf16 ones-matmuls) + chain -> rb_h, mrow_h
  2. prefetch x(t+1): xeff, LN_x stats + chain (same ACT table set), zx
  3. zh = heff*rb_h; (bwd: heff += spat*(tpos[t-1]-tpos[t]) in place)
  4. q/k/v matmuls; mean-correction folded in as one rank-1 matmul
     against [ones; mrb_x; mrb_h]; elu via exp
  5. kv-state matmuls; pack diag blocks fp16 (x1/256); AllReduce
  6. deferred MLP of frame t-1 (fp8 e4m3 DoubleRow matmuls, gelu)
  7. kv back -> block-diag via 2 DMAs; o = kv@q; attn = (256*Wout)@o;
     epilogue: heff_next = ps+bout+heff, x2 = ps+bout+xeff
  8. LN_o(t) stats + chain (rsqrt set, adjacent to next frame's step 1);
     z2 -> fp8

The carry is heff = h + pos (not h): forward scans use a fixed pos so
heff_next = attn + heff directly; backward scans adjust by the host-
precomputed tpos delta.

Precision: residual/carry fp32; LN'd activations fp16; kv state fp16 at
1/256; stats moving operands f32r (sum) / bf16 (sumsq); MLP weights and
activations fp8 e4m3 (the MLP output is ~300x smaller than the residual
stream, so fp8 noise is diluted well below tolerance); 1/sqrt via the
Abs_reciprocal_sqrt activation.
"""

import functools

import numpy as np
import ml_dtypes

import concourse.bacc as bacc
import concourse.mybir as mybir
from concourse import tile
from concourse.bass_utils import run_bass_kernel_spmd

F32 = mybir.dt.float32
F32R = mybir.dt.float32r
BF16 = mybir.dt.bfloat16
F16 = mybir.dt.float16
F8 = mybir.dt.float8e4
AF = mybir.ActivationFunctionType
ALU = mybir.AluOpType
NPF8 = ml_dtypes.float8_e4m3

B, L, N, D, M, H = 2, 12, 1024, 768, 3072, 12
NCORES = 8
GROUP = 4                 # cores per batch group
TOK = N // GROUP          # 256 tokens per core
KT = D // 128             # 6 d-tiles
KT3 = KT // 2             # 3 double-row k-groups
MT = M // 128             # 24 m-tiles
MT2 = MT // 2             # 12 double-row m-groups
F3 = 3 * D                # 2304
EPS = 1e-5
KVS = 1.0 / 256.0         # kv-state scale so fp16 holds it (256 in Wout)

# dev-scale knobs (full problem: L_RUN=12, LAYERS_RUN=2, DIRS_RUN=(0, 1))
L_RUN = L
LAYERS_RUN = 2
DIRS_RUN = (0, 1)

REPLICA_GROUPS = [[0, 1, 2, 3], [4, 5, 6, 7]]


# ---------------------------------------------------------------- host prep

def _pack_weights(inputs):
    segs = []
    for layer in range(LAYERS_RUN):
        for d in DIRS_RUN:
            gi = np.asarray(inputs["lni_g"][d, layer]); bi = np.asarray(inputs["lni_b"][d, layer])
            gh = np.asarray(inputs["lnh_g"][d, layer]); bh = np.asarray(inputs["lnh_b"][d, layer])
            go = np.asarray(inputs["lno_g"][d, layer]); bo = np.asarray(inputs["lno_b"][d, layer])
            Wqkv = np.asarray(inputs["Wqkv"][d, layer]); bqkv = np.asarray(inputs["bqkv"][d, layer])
            Wqkvh = np.asarray(inputs["Wqkvh"][d, layer]); bqkvh = np.asarray(inputs["bqkvh"][d, layer])
            Wout = np.asarray(inputs["Wout"][d, layer]); bout = np.asarray(inputs["bout"][d, layer])
            W1 = np.asarray(inputs["W1"][d, layer]); b1 = np.asarray(inputs["b1"][d, layer])
            W2 = np.asarray(inputs["W2"][d, layer]); b2 = np.asarray(inputs["b2"][d, layer])

            gq = gi[:, None] * Wqkv                        # (D, 3D)
            gqh = gh[:, None] * Wqkvh
            cq = bi @ Wqkv + bqkv + bh @ Wqkvh + bqkvh     # (3D,)
            # rank-1 mean-correction rows:
            # qkv = zx'@gq + zh'@gqh + 1*cq - mrb_x*colsum(gq) - mrb_h*colsum(gqh)
            qcF = np.zeros((4, F3), np.float32)
            qcF[0] = cq
            qcF[1] = -gq.sum(axis=0)
            qcF[2] = -gqh.sum(axis=0)

            g1 = go[:, None] * W1                          # (D, M)
            c1 = bo @ W1 + b1                              # (M,)

            # fp8 DoubleRow layouts, quartered for streaming
            # g1q[q, p, mjl, kd3, j, c] = g1[kd3*256 + j*128 + p, (q*6+mjl)*128 + c]
            g1t = g1.reshape(KT3, 2, 128, 4, 6, 128)       # [kd3, j, p, q, mjl, c]
            g1q = np.ascontiguousarray(
                g1t.transpose(3, 2, 4, 0, 1, 5)).astype(NPF8)  # (4,128,6,KT3,2,128)
            # w2q[wh, p, mj2l, fw, j, c] = W2[(half*6+mj2l)*256 + j*128 + p,
            #                                 (wv*3+fw)*128 + c];  wh = wv*2+half
            w2t = W2.reshape(2, 6, 2, 128, 2, 3, 128)      # [half, mj2l, j, p, wv, fw, c]
            w2q = np.ascontiguousarray(
                w2t.transpose(4, 0, 3, 1, 5, 2, 6)         # [wv, half, p, mj2l, fw, j, c]
                .reshape(4, 128, 6, 3, 2, 128)).astype(NPF8)

            seg = dict(
                gqkv=np.ascontiguousarray(
                    gq.reshape(KT, 128, F3).transpose(1, 0, 2)).astype(np.float16),
                gqkvh=np.ascontiguousarray(
                    gqh.reshape(KT, 128, F3).transpose(1, 0, 2)).astype(np.float16),
                qcf=qcF.astype(np.float16),                 # (4, F3)
                wout=np.ascontiguousarray(
                    (Wout * 256.0).reshape(KT, 128, D).transpose(1, 0, 2)).astype(np.float16),
                bout=np.ascontiguousarray(
                    bout.reshape(KT, 128).T).astype(np.float32),      # (128, KT)
                g1q=g1q,
                c1=np.ascontiguousarray(
                    c1.reshape(MT, 128).T).astype(np.float32),        # (128, MT)
                w2q=w2q,
                b2=np.ascontiguousarray(
                    b2.reshape(KT, 128).T).astype(np.float32),        # (128, KT)
            )
            segs.append(seg)
    return segs


def _feat_major(a, dtype):
    """(..., tok, D) -> (..., 128, KT, tok) tiled feature-major."""
    t = np.moveaxis(np.asarray(a), -1, -2)                # (..., D, tok)
    shp = t.shape[:-2]
    t = t.reshape(shp + (KT, 128, t.shape[-1]))           # (..., KT, 128, tok)
    t = np.moveaxis(t, -3, -2)                            # (..., 128, KT, tok)
    return np.ascontiguousarray(t).astype(dtype)


def make_in_maps(inputs):
    segs = _pack_weights(inputs)
    in_maps = []
    for core in range(NCORES):
        b = core // GROUP
        s = (core % GROUP) * TOK
        m = {}
        m["x_in"] = _feat_major(
            np.asarray(inputs["x"])[b, :L_RUN, s:s + TOK, :], np.float32)
        m["h0_in"] = _feat_major(
            np.asarray(inputs["hidden"])[b, s:s + TOK, :], np.float32)
        m["spat"] = _feat_major(
            np.asarray(inputs["spatial_pos"])[b, s:s + TOK, :], np.float32)
        tp = np.asarray(inputs["temporal_pos"])[b, :L_RUN, :]   # (L, D)
        tp = tp.T.reshape(KT, 128, L_RUN).transpose(1, 0, 2)
        tp = np.ascontiguousarray(tp).astype(np.float32)        # (128, KT, L)
        m["tpos"] = tp
        tpd = np.zeros_like(tp)
        tpd[:, :, 1:] = tp[:, :, :-1] - tp[:, :, 1:]            # bwd pos delta
        m["tpd"] = tpd
        for si, seg in enumerate(segs):
            for k, v in seg.items():
                m[f"{k}_{si}"] = v
        in_maps.append(m)
    return in_maps


def unshard_output(results):
    out = np.empty((B, L_RUN, N, D), np.float32)
    for core in range(NCORES):
        b = core // GROUP
        s = (core % GROUP) * TOK
        o = np.asarray(results[core]["out_x"])            # (L, 128, KT, TOK)
        o = o.transpose(0, 2, 1, 3).reshape(L_RUN, D, TOK)
        out[b, :, s:s + TOK, :] = np.moveaxis(o, -1, -2)
    return out


# ---------------------------------------------------------------- kernel build

class Ctx:
    pass


def _r(ap):
    return ap.bitcast(F32R)


def _ln_stats(nc, cx, src32):
    """s1/s2 psum tiles for a feature-major (128, KT, TOK) f32 tile.
    Sums over the feature (partition) axis via all-ones bf16 matmuls;
    the bf16 cast of the source runs on the otherwise-idle GpSimd."""
    s1 = cx.psA.tile([128, TOK], F32, name="ps", tag="ps")
    s2 = cx.psA.tile([128, TOK], F32, name="ps", tag="ps")
    for kd in range(KT):
        xb = cx.sqp.tile([128, TOK], BF16, name="xb", tag="xb")
        nc.gpsimd.tensor_copy(xb[:], src32[:, kd, :])
        sq = cx.sqp.tile([128, TOK], BF16, name="sq", tag="sq")
        nc.scalar.activation(sq[:], src32[:, kd, :], AF.Square)
        nc.tensor.matmul(s1[:], cx.onesB[:], xb[:],
                         start=(kd == 0), stop=(kd == KT - 1))
        nc.tensor.matmul(s2[:], cx.onesB[:], sq[:],
                         start=(kd == 0), stop=(kd == KT - 1))
    return s1, s2


def _ln_chain(nc, cx, s1, s2, tag, mrow=None):
    """rb (128, TOK) f32 from the stats psums; optionally writes mean*rb
    fp16 into `mrow` ((1, TOK) AP). Returns (rb, mean)."""
    mean = cx.tmp1.tile([128, TOK], F32, name=f"mean_{tag}", tag=f"mean_{tag}")
    nc.vector.tensor_scalar_mul(mean[:], s1[:], 1.0 / D)
    t1 = cx.tmp1.tile([128, TOK], F32, name=f"t1_{tag}", tag=f"t1_{tag}")
    nc.vector.tensor_mul(t1[:], mean[:], s1[:])           # = D*mean^2
    vr = cx.tmp1.tile([128, TOK], F32, name=f"vr_{tag}", tag=f"vr_{tag}")
    nc.vector.tensor_sub(vr[:], s2[:], t1[:])             # = D*var
    rb = cx.tmp1.tile([128, TOK], F32, name=f"rb_{tag}", tag=f"rb_{tag}")
    nc.scalar.activation(rb[:], vr[:], AF.Abs_reciprocal_sqrt,
                         scale=1.0 / D, bias=cx.epsc[:])
    if mrow is not None:
        # engines can't address a 1-partition AP off base 0: stage the
        # row at partition 0 and DMA it into place
        mr0 = cx.tmp1.tile([1, TOK], F16, name=f"mr_{tag}", tag=f"mr_{tag}")
        nc.vector.tensor_mul(mr0[:], mean[0:1, :], rb[0:1, :])
        nc.sync.dma_start(mrow, mr0[:])
    return rb, mean


def _elu1(nc, cx, psum_ap, out_ap, ncols):
    """out = elu(psum)+1 = exp(min(x,0)) + max(x,0)."""
    tmin = cx.tmp2.tile([128, 384], F32, name="emin", tag="emin")
    texp = cx.tmp2.tile([128, 384], F32, name="eexp", tag="eexp")
    nc.vector.tensor_scalar_min(tmin[:, :ncols], psum_ap, 0.0)
    nc.scalar.activation(texp[:, :ncols], tmin[:, :ncols], AF.Exp)
    nc.vector.scalar_tensor_tensor(out_ap, psum_ap, 0.0, texp[:, :ncols],
                                   op0=ALU.max, op1=ALU.add)


def build_nc():
    nc = bacc.Bacc("TRN2", target_bir_lowering=False, debug=False,
                   num_devices=NCORES)

    x_in = nc.dram_tensor("x_in", [L_RUN, 128, KT, TOK], F32, kind="ExternalInput")
    h0_in = nc.dram_tensor("h0_in", [128, KT, TOK], F32, kind="ExternalInput")
    spat = nc.dram_tensor("spat", [128, KT, TOK], F32, kind="ExternalInput")
    tpos = nc.dram_tensor("tpos", [128, KT, L_RUN], F32, kind="ExternalInput")
    tpd = nc.dram_tensor("tpd", [128, KT, L_RUN], F32, kind="ExternalInput")
    nseg = LAYERS_RUN * len(DIRS_RUN)
    segs = []
    for si in range(nseg):
        segs.append(dict(
            gqkv=nc.dram_tensor(f"gqkv_{si}", [128, KT, F3], F16, kind="ExternalInput"),
            gqkvh=nc.dram_tensor(f"gqkvh_{si}", [128, KT, F3], F16, kind="ExternalInput"),
            qcf=nc.dram_tensor(f"qcf_{si}", [4, F3], F16, kind="ExternalInput"),
            wout=nc.dram_tensor(f"wout_{si}", [128, KT, D], F16, kind="ExternalInput"),
            bout=nc.dram_tensor(f"bout_{si}", [128, KT], F32, kind="ExternalInput"),
            g1q=nc.dram_tensor(f"g1q_{si}", [4, 128, 6, KT3, 2, 128], F8, kind="ExternalInput"),
            c1=nc.dram_tensor(f"c1_{si}", [128, MT], F32, kind="ExternalInput"),
            w2q=nc.dram_tensor(f"w2q_{si}", [4, 128, 6, 3, 2, 128], F8, kind="ExternalInput"),
            b2=nc.dram_tensor(f"b2_{si}", [128, KT], F32, kind="ExternalInput"),
        ))
    out_x = nc.dram_tensor("out_x", [L_RUN, 128, KT, TOK], F32, kind="ExternalOutput")

    with tile.TileContext(nc) as tc:
        with (
            tc.tile_pool(name="cst", bufs=1) as cst,
            tc.tile_pool(name="wt", bufs=1) as wt,
            tc.tile_pool(name="stream", bufs=2) as stream,
            tc.tile_pool(name="actA", bufs=2) as actA,
            tc.tile_pool(name="actB", bufs=1) as actB,
            tc.tile_pool(name="state", bufs=1) as state,
            tc.tile_pool(name="tmp1", bufs=1) as tmp1,
            tc.tile_pool(name="tmp2", bufs=2) as tmp2,
            tc.tile_pool(name="sqp", bufs=3) as sqp,
            tc.tile_pool(name="psA", bufs=5, space="PSUM") as psA,
            tc.tile_pool(name="psY", bufs=3, space="PSUM") as psY,
            tc.tile_pool(name="dram", bufs=4, space="DRAM") as dram,
        ):
            cx = Ctx()
            cx.wt, cx.stream, cx.actA, cx.actB = wt, stream, actA, actB
            cx.state, cx.tmp1, cx.tmp2, cx.sqp = state, tmp1, tmp2, sqp
            cx.psA, cx.psY, cx.dram = psA, psY, dram
            cx.h0_in, cx.tpd = h0_in, tpd

            cx.onesB = cst.tile([128, 128], BF16, name="onesB")
            nc.vector.memset(cx.onesB[:], 1.0)
            cx.epsc = cst.tile([128, 1], F32, name="epsc")
            nc.vector.memset(cx.epsc[:], EPS)
            cx.spat = cst.tile([128, KT, TOK], F32, name="spatc")
            nc.sync.dma_start(cx.spat[:], spat.ap())
            cx.tpos = cst.tile([128, KT, L_RUN], F32, name="tposc")
            nc.sync.dma_start(cx.tpos[:], tpos.ap())
            cx.tpdc = cst.tile([128, KT, L_RUN], F32, name="tpdc")
            nc.sync.dma_start(cx.tpdc[:], tpd.ap())
            # block-diag kv holder: off-diagonal blocks stay zero forever
            cx.bd16 = state.tile([128, KT, 128], F16, name="bd16", tag="bd16")
            nc.vector.memset(cx.bd16[:], 0.0)
            # per-parity [ones; mrb_x; mrb_h] rows for the bias/correction mm
            cx.mrows = []
            for p in range(2):
                mr = state.tile([4, TOK], F16, name=f"mrows{p}", tag=f"mrows{p}")
                nc.vector.memset(mr[0:1, :], 1.0)
                cx.mrows.append(mr)

            x1_sc = dram.tile([L_RUN, 128, KT, TOK], F32, name="x1_sc", tag="x1_sc")
            yf_sc = dram.tile([L_RUN, 128, KT, TOK], F32, name="yf_sc", tag="yf_sc")

            for layer in range(LAYERS_RUN):
                x_src = x_in.ap() if layer == 0 else x1_sc
                last_layer = layer == LAYERS_RUN - 1
                for dir_i, d in enumerate(DIRS_RUN):
                    si = layer * len(DIRS_RUN) + dir_i
                    fwd = d == 0
                    last_scan = dir_i == len(DIRS_RUN) - 1
                    frames = (list(range(L_RUN)) if fwd
                              else list(range(L_RUN - 1, -1, -1)))
                    if not last_scan:
                        out_dst = yf_sc
                    elif last_layer:
                        out_dst = out_x.ap()
                    else:
                        out_dst = x1_sc
                    _emit_scan(nc, cx, segs[si], x_src, frames,
                               pos_fixed=(layer if fwd else None),
                               yf_sc=yf_sc, fwd=fwd, out_dst=out_dst)
    nc.compile()
    return nc


def _emit_scan(nc, cx, seg, x_src, frames, pos_fixed, yf_sc, fwd, out_dst):
    w = {}
    for nm, shape, dt in (("gqkv", [128, KT, F3], F16),
                          ("gqkvh", [128, KT, F3], F16),
                          ("qcf", [4, F3], F16),
                          ("wout", [128, KT, D], F16),
                          ("bout", [128, KT], F32),
                          ("c1", [128, MT], F32),
                          ("b2", [128, KT], F32)):
        w[nm] = cx.wt.tile(shape, dt, name=nm, tag=nm)
        nc.sync.dma_start(w[nm][:], seg[nm].ap())
    w["g1q"] = seg["g1q"]
    w["w2q"] = seg["w2q"]

    # h carry as heff = h + pos_tp (f32)
    tp0 = pos_fixed if pos_fixed is not None else frames[0]
    heff = cx.actA.tile([128, KT, TOK], F32, name="heff", tag="heff")
    nc.sync.dma_start(heff[:], cx.h0_in.ap())
    for kd in range(KT):
        nc.vector.scalar_tensor_tensor(
            heff[:, kd, :], cx.spat[:, kd, :], cx.tpos[:, kd, tp0:tp0 + 1],
            heff[:, kd, :], op0=ALU.mult, op1=ALU.add)

    xpre = _emit_xside(nc, cx, x_src, frames[0], parity=0)

    pend = None
    for i, t in enumerate(frames):
        nxt = frames[i + 1] if i + 1 < len(frames) else None
        heff, xpre, pend = _emit_frame(
            nc, cx, w, t, nxt, x_src, heff, xpre, pos_fixed, yf_sc,
            fwd, out_dst, pend, parity=i % 2)
    _emit_mlp(nc, cx, w, pend)


def _emit_xside(nc, cx, x_src, t, parity):
    """xeff = x_t + pos_t; LN_x stats+chain; zx = xeff*rb; mrow_x."""
    xeff = cx.actA.tile([128, KT, TOK], F32, name="xe", tag="xe")
    nc.sync.dma_start(xeff[:], x_src[t])
    for kd in range(KT):
        nc.vector.scalar_tensor_tensor(
            xeff[:, kd, :], cx.spat[:, kd, :], cx.tpos[:, kd, t:t + 1],
            xeff[:, kd, :], op0=ALU.mult, op1=ALU.add)
    s1, s2 = _ln_stats(nc, cx, xeff)
    rb, _ = _ln_chain(nc, cx, s1, s2, "x", mrow=cx.mrows[parity][1:2, :])
    zx = cx.actA.tile([128, KT, TOK], F16, name="zx", tag="zx")
    for kd in range(KT):
        nc.vector.tensor_mul(zx[:, kd, :], xeff[:, kd, :], rb[:])
    return dict(xeff=xeff, zx=zx)


def _emit_frame(nc, cx, w, t, nxt, x_src, heff, xpre, pos_fixed, yf_sc,
                fwd, out_dst, pend, parity):
    mrows = cx.mrows[parity]
    xeff, zx = xpre["xeff"], xpre["zx"]

    # ---- 1. LN_h
    s1h, s2h = _ln_stats(nc, cx, heff)
    rb_h, _ = _ln_chain(nc, cx, s1h, s2h, "h", mrow=mrows[2:3, :])

    # ---- 2. prefetch next frame's x-side (writes mrows[1-parity][1])
    xpre_n = (_emit_xside(nc, cx, x_src, nxt, parity=1 - parity)
              if nxt is not None else None)

    # ---- 3. zh; bwd pos adjustment of the carry (in place, on gpsimd)
    zh = cx.actB.tile([128, KT, TOK], F16, name="zh", tag="zh")
    for kd in range(KT):
        nc.vector.tensor_mul(zh[:, kd, :], heff[:, kd, :], rb_h[:])
    if pos_fixed is None and nxt is not None:
        for kd in range(KT):
            nc.vector.scalar_tensor_tensor(
                heff[:, kd, :], cx.spat[:, kd, :], cx.tpdc[:, kd, t:t + 1],
                heff[:, kd, :], op0=ALU.mult, op1=ALU.add)

    # ---- 4. q (feature-major)
    q16 = cx.actB.tile([128, KT, TOK], F16, name="q16", tag="q16")
    for ft in range(KT):
        ps = cx.psA.tile([128, TOK], F32, name="ps", tag="ps")
        lo = ft * 128
        for kd in range(KT):
            nc.tensor.matmul(ps[:], w["gqkv"][:, kd, lo:lo + 128],
                             zx[:, kd, :], start=(kd == 0), stop=False)
        for kd in range(KT):
            nc.tensor.matmul(ps[:], w["gqkvh"][:, kd, lo:lo + 128],
                             zh[:, kd, :], start=False, stop=False)
        nc.tensor.matmul(ps[:], w["qcf"][0:3, lo:lo + 128],
                         mrows[0:3, :], start=False, stop=True)
        _elu1(nc, cx, ps[:], q16[:, ft, :], TOK)

    # ---- k, v (token-major): (128, 2, D) each [tok-half, feature]
    k16 = cx.actB.tile([128, 2, D], F16, name="k16", tag="k16")
    v16 = cx.actB.tile([128, 2, D], F16, name="v16", tag="v16")
    for tok2 in range(2):
        ts = slice(tok2 * 128, (tok2 + 1) * 128)
        for fc in range(4):  # chunks of 384: k then v
            lo = D + fc * 384
            ps = cx.psA.tile([128, 384], F32, name="ps", tag="ps")
            for kd in range(KT):
                nc.tensor.matmul(ps[:], zx[:, kd, ts],
                                 w["gqkv"][:, kd, lo:lo + 384],
                                 start=(kd == 0), stop=False)
            for kd in range(KT):
                nc.tensor.matmul(ps[:], zh[:, kd, ts],
                                 w["gqkvh"][:, kd, lo:lo + 384],
                                 start=False, stop=False)
            nc.tensor.matmul(ps[:], mrows[0:3, ts],
                             w["qcf"][0:3, lo:lo + 384], start=False, stop=True)
            off = fc * 384
            if fc < 2:
                _elu1(nc, cx, ps[:], k16[:, tok2, off:off + 384], 384)
            else:
                nc.scalar.activation(v16[:, tok2, off - D:off - D + 384],
                                     ps[:], AF.Copy)

    # ---- 5. kv state per head-pair; pack diag blocks fp16 (x KVS)
    kvpk = cx.actB.tile([128, KT, 64], F16, name="kvpk", tag="kvpk")
    for hp in range(KT):
        ps = cx.psA.tile([128, 384], F32, name="ps", tag="ps")
        pskv = ps[:, 0:128]
        hs = slice(hp * 128, (hp + 1) * 128)
        for tok2 in range(2):
            nc.tensor.matmul(pskv, k16[:, tok2, hs], v16[:, tok2, hs],
                             start=(tok2 == 0), stop=(tok2 == 1))
        nc.scalar.activation(kvpk[0:64, hp, :], pskv[0:64, 0:64],
                             AF.Copy, scale=KVS)
        nc.scalar.activation(kvpk[64:128, hp, :], pskv[64:128, 64:128],
                             AF.Copy, scale=KVS)

    arin = cx.dram.tile([128, KT, 64], F16, name="arin", tag="arin")
    arout = cx.dram.tile([128, KT, 64], F16, name="arout", tag="arout")
    nc.sync.dma_start(arin[:], kvpk[:])
    nc.gpsimd.collective_compute(
        "AllReduce", ALU.add, replica_groups=REPLICA_GROUPS,
        ins=[arin.opt()], outs=[arout.opt()])

    # ---- 6. deferred MLP of the previous frame (hides the all-reduce)
    if pend is not None:
        _emit_mlp(nc, cx, w, pend)

    # ---- 7. kv back -> block-diag via 2 DMAs; o; attn; epilogue
    nc.sync.dma_start(cx.bd16[0:64, :, 0:64], arout[0:64])
    nc.sync.dma_start(cx.bd16[64:128, :, 64:128], arout[64:128])
    o16 = cx.actB.tile([128, KT, TOK], F16, name="o16", tag="o16")
    for hp in range(KT):
        ps = cx.psA.tile([128, TOK], F32, name="ps", tag="ps")
        nc.tensor.matmul(ps[:], cx.bd16[:, hp, :], q16[:, hp, :],
                         start=True, stop=True)
        nc.scalar.activation(o16[:, hp, :], ps[:], AF.Copy)

    heff_n = cx.actA.tile([128, KT, TOK], F32, name="heff", tag="heff")
    x232 = cx.actA.tile([128, KT, TOK], F32, name="x232", tag="x232")
    for ft in range(KT):
        ps = cx.psA.tile([128, TOK], F32, name="ps", tag="ps")
        for hp in range(KT):
            nc.tensor.matmul(ps[:], w["wout"][:, hp, ft * 128:(ft + 1) * 128],
                             o16[:, hp, :], start=(hp == 0), stop=(hp == KT - 1))
        bo = w["bout"][:, ft:ft + 1]
        nc.vector.scalar_tensor_tensor(
            heff_n[:, ft, :], ps[:], bo, heff[:, ft, :],
            op0=ALU.add, op1=ALU.add)
        nc.vector.scalar_tensor_tensor(
            x232[:, ft, :], ps[:], bo, xeff[:, ft, :],
            op0=ALU.add, op1=ALU.add)

    # ---- 8. LN_o; z2 -> fp8 (consumed by the deferred MLP next frame)
    s1o, s2o = _ln_stats(nc, cx, x232)
    rb2, mean_o = _ln_chain(nc, cx, s1o, s2o, "o")
    mrb2 = cx.tmp1.tile([128, TOK], F32, name="mrb2", tag="mrb2")
    nc.vector.tensor_mul(mrb2[:], mean_o[:], rb2[:])
    z2 = cx.actA.tile([128, KT3, 2, TOK], F8, name="z2", tag="z2")
    for kd in range(KT):
        zt = cx.tmp2.tile([128, TOK], F32, name="zt", tag="zt")
        nc.gpsimd.tensor_mul(zt[:], x232[:, kd, :], rb2[:])
        nc.vector.tensor_sub(z2[:, kd // 2, kd % 2, :], zt[:], mrb2[:])

    pend = dict(t=t, z2=z2, x232=x232, fwd=fwd, out_dst=out_dst, yf_sc=yf_sc)
    return heff_n, xpre_n, pend


def _emit_mlp(nc, cx, w, pend):
    t, z2, x232 = pend["t"], pend["z2"], pend["x232"]
    fwd, out_dst, yf_sc = pend["fwd"], pend["out_dst"], pend["yf_sc"]

    # y1 = gelu(z2 @ G1 + c1) -> fp8, m on partitions; G1 streamed by quarter
    y1g = cx.actB.tile([128, MT2, 2, TOK], F8, name="y1g", tag="y1g")
    for q in range(4):
        g1s = cx.stream.tile([128, 6, KT3, 2, 128], F8, name="g1s", tag="g1s")
        nc.sync.dma_start(g1s[:], w["g1q"].ap()[q])
        for mjl in range(6):
            mj = q * 6 + mjl
            ps = cx.psA.tile([128, TOK], F32, name="ps", tag="ps")
            for kd3 in range(KT3):
                nc.tensor.matmul(ps[:], g1s[:, mjl, kd3, :, :],
                                 z2[:, kd3, :, :], start=(kd3 == 0),
                                 stop=(kd3 == KT3 - 1),
                                 perf_mode=mybir.MatmulPerfMode.DoubleRow)
            nc.scalar.activation(y1g[:, mj // 2, mj % 2, :], ps[:], AF.Gelu,
                                 bias=w["c1"][:, mj:mj + 1])

    # y = y1 @ W2 (+ b2); out = x2 + y  (two waves of 3 PSUM banks;
    # W2 streamed by (wave, mj2-half) quarter)
    outt = cx.actB.tile([128, KT, TOK], F32, name="outt", tag="outt")
    if not fwd:
        yf = cx.actB.tile([128, KT, TOK], F32, name="yfld", tag="yfld")
        nc.sync.dma_start(yf[:], yf_sc[t])
    for wv in range(2):
        yps = [cx.psY.tile([128, TOK], F32, name="psy", tag="psy")
               for _ in range(3)]
        for half in range(2):
            w2s = cx.stream.tile([128, 6, 3, 2, 128], F8, name="w2s", tag="w2s")
            nc.sync.dma_start(w2s[:], w["w2q"].ap()[wv * 2 + half])
            for fw in range(3):
                for mj2l in range(6):
                    mj2 = half * 6 + mj2l
                    nc.tensor.matmul(yps[fw][:], w2s[:, mj2l, fw, :, :],
                                     y1g[:, mj2, :, :],
                                     start=(mj2 == 0), stop=(mj2 == MT2 - 1),
                                     perf_mode=mybir.MatmulPerfMode.DoubleRow)
        for fw in range(3):
            ft = wv * 3 + fw
            nc.vector.scalar_tensor_tensor(
                outt[:, ft, :], yps[fw][:], w["b2"][:, ft:ft + 1],
                x232[:, ft, :], op0=ALU.add, op1=ALU.add)
            if not fwd:
                nc.vector.tensor_add(outt[:, ft, :], outt[:, ft, :],
                                     yf[:, ft, :])
    nc.sync.dma_start(out_dst[t], outt[:])


# ---------------------------------------------------------------- entry point

@functools.cache
def _compiled_nc():
    return build_nc()


def kernel(**inputs):
    inputs = {k: np.asarray(v) for k, v in inputs.items()}
    nc = _compiled_nc()
    in_maps = make_in_maps(inputs)
    res = run_bass_kernel_spmd(nc, in_maps, list(range(NCORES)))
    return unshard_output(res.results)
